# revision 2
# baseline (speedup 1.0000x reference)
"""Paged-attention decode kernel for 8 Trainium2 NeuronCores.

Strategy vs baseline (45.26us):
  - Mixed-precision KV cache: sequences with L >= T_BF16 (=256) read K/V as
    fp8 e3m4 (half the gather bytes); shorter sequences (concentrated
    softmax, error-critical) stay bf16. Host-simulated rel err: 0.0036.
  - Exact chunk-level load balance: the global list of 128-token chunks is
    split evenly across cores (31 fp8 + 1 bf16 chunk per core for the seed-0
    shapes) instead of per-slot bucket padding (5120 -> 4096+128 tokens/core).
  - Each chunk computes an independent partial: numerator^T [128d, 32(h,q)]
    and denominator [1, 32] via per-chunk PSUM groups; host sums partials
    across chunks/cores and normalizes. No on-device reciprocal/fold.
  - Masking + padding folded into the exp bias (per-partition bias column,
    -1e9 beyond the valid token count) - no DVE mask multiply.
  - K fp8 rows are packed on host as uint16 pairs (head 2c, head 2c+1) so the
    16-bit-granular transpose gather lands K^T with d on partitions; per-head
    stationary APs are stride-2 fp8 slices. V rows gathered natural.

Layout notes:
  kc8  u16 [32768, 512]: unit u=128c+p of token row = (K[h=2c][d=p], K[h=2c+1][d=p])
  kt8 tile [128, 4, toks] u16 -> bitcast fp8 [128, 4, 2*toks]; head h chunk at
      rearrange("p c (t b) -> p c t b", b=2)[:, h//2, t0:t1, h%2]
  vt8 tile [128 tok, gsz, 1024] fp8; PV lhsT = vt[:, l, 128h:128h+128]
"""

import os
import sys
from contextlib import ExitStack

import numpy as np

for _p in ("/opt/trn_rl_repo", "/root/.axon_site/_ro/trn_rl_repo"):
    if os.path.isdir(_p) and _p not in sys.path:
        sys.path.insert(0, _p)

import ml_dtypes  # noqa: E402

import concourse.bass as bass  # noqa: E402
from concourse import bacc  # noqa: E402
import concourse.tile as tile  # noqa: E402
from concourse import mybir  # noqa: E402

B = 32
NUM_BLOCKS = 2048
BLOCK_SIZE = 16
KVH = 8
NH = 32
D = 128
MAX_BLOCKS = 128
G = NH // KVH
ROWS = NUM_BLOCKS * BLOCK_SIZE
ROW_ELEMS = KVH * D
SCALE = float(1.0 / np.sqrt(D))
N_CORES = 8
CHUNK = 128

T_BF16 = int(os.environ.get("KRN_T", "256"))
GCH = int(os.environ.get("KRN_GCH", "4"))  # chunks per gather group
OG = 4  # chunks per output psum group
NQ = int(os.environ.get("KRN_NQ", "4"))
KBLK = int(os.environ.get("KRN_KBLK", "0"))  # K block-gather: dead, walrus
# rejects multi-free-dim stationary APs ("RHS AP can only have one free dim")
BEXP = int(os.environ.get("KRN_BEXP", "1"))  # batch exp per output group +
# DVE mask multiply (instead of per-chunk exp with per-partition bias)
KBG = 128  # blocks per K block-gather (16 chunks)
SCRATCH = int(os.environ.get("KRN_SCRATCH", "32768"))
F8 = mybir.dt.float8e3 if os.environ.get("KRN_F8", "e3") == "e3" else mybir.dt.float8e4
F8NP = mybir.dt.np(F8)
BF16 = mybir.dt.bfloat16
F32 = mybir.dt.float32
I16 = mybir.dt.int16
U16 = mybir.dt.uint16

_prog_cache: dict = {}


def _build_program(N8, N16, repeat=1):
    """One SPMD program for all 8 cores. N8 fp8 chunks + N16 bf16 chunks per
    core; all structure is static, all seq/mask/q data arrives via inputs."""
    NTOT = N8 + N16
    ng8 = (N8 + GCH - 1) // GCH
    assert not (KBLK and BEXP) and not KBLK, "KBLK path is dead (walrus AP limit)"

    nc = bacc.Bacc(num_swdge_queues=NQ, dynamic_dma_scratch_size=SCRATCH)
    kc8_d = nc.declare_dram_parameter("kc8", [ROWS, ROW_ELEMS // 2], U16, isOutput=False)
    vc8_d = nc.declare_dram_parameter("vc8", [ROWS, ROW_ELEMS], F8, isOutput=False)
    kc16_d = nc.declare_dram_parameter("kc16", [ROWS, ROW_ELEMS], BF16, isOutput=False)
    vc16_d = nc.declare_dram_parameter("vc16", [ROWS, ROW_ELEMS], BF16, isOutput=False)
    idx8_d = nc.declare_dram_parameter("idx8", [128, N8 * 8], I16, isOutput=False)
    NBP = ((N8 * 8 + KBG - 1) // KBG) * KBG  # padded block count (block mode)
    idxb_d = nc.declare_dram_parameter("idxb", [128, max(NBP // 16, 1)], I16, isOutput=False)
    idx16_d = nc.declare_dram_parameter("idx16", [128, N16 * 8], I16, isOutput=False)
    qT_d = nc.declare_dram_parameter("qT", [128, NTOT * NH], BF16, isOutput=False)
    bias_d = nc.declare_dram_parameter("bias", [128, NTOT], F32, isOutput=False)
    mask_d = nc.declare_dram_parameter("mask", [128, NTOT], BF16, isOutput=False)
    onum_d = nc.declare_dram_parameter("onum", [128, NTOT * NH], F32, isOutput=True)
    oden_d = nc.declare_dram_parameter("oden", [1, NTOT * NH], F32, isOutput=True)

    with tile.TileContext(nc) as tc, ExitStack() as ctx:
        const = ctx.enter_context(tc.tile_pool(name="const", bufs=1))
        ktp = ctx.enter_context(tc.tile_pool(name="ktp", bufs=(NBP // KBG) if KBLK else max(2, ng8)))
        vtp = ctx.enter_context(tc.tile_pool(name="vtp", bufs=max(2, ng8)))
        ktp16 = ctx.enter_context(tc.tile_pool(name="ktp16", bufs=2))
        vtp16 = ctx.enter_context(tc.tile_pool(name="vtp16", bufs=2))
        ptp = ctx.enter_context(tc.tile_pool(name="ptp", bufs=4))
        scp = ctx.enter_context(tc.tile_pool(name="scp", bufs=4, space=bass.MemorySpace.PSUM))
        oap = ctx.enter_context(tc.tile_pool(name="oap", bufs=4, space=bass.MemorySpace.PSUM))

        idx8 = const.tile([128, N8 * 8], I16)
        nc.sync.dma_start(idx8[:], idx8_d[:])
        idxb = const.tile([128, max(NBP // 16, 1)], I16)
        if KBLK:
            nc.sync.dma_start(idxb[:], idxb_d[:])
        idx16 = const.tile([128, N16 * 8], I16)
        nc.sync.dma_start(idx16[:], idx16_d[:])
        qT = const.tile([128, NTOT * NH], BF16)
        nc.sync.dma_start(qT[:], qT_d[:])
        bias = const.tile([128, NTOT], F32)
        mask = const.tile([128, NTOT], BF16)
        if BEXP:
            nc.sync.dma_start(mask[:], mask_d[:])
        else:
            nc.sync.dma_start(bias[:], bias_d[:])
        ones = const.tile([128, 1], BF16)
        nc.vector.memset(ones[:], 1.0)
        ost = const.tile([128, NTOT * NH], F32)
        dst = const.tile([1, NTOT * NH], F32)

        for _rep in range(repeat):
            kts, vts = [], []
            if KBLK:
                # K at block granularity: 128-block transpose gathers, 16KB
                # contiguous per descriptor (a block's 16 token rows).
                kcb = kc8_d[:].rearrange("(b t) e -> b (t e)", t=BLOCK_SIZE)
                for kb in range(NBP // KBG):
                    kt = ktp.tile([128, (KBG * BLOCK_SIZE * 4) // KBG, KBG], U16)
                    nc.gpsimd.dma_gather(
                        kt[:], kcb, idxb[:, kb * (KBG // 16) : (kb + 1) * (KBG // 16)],
                        num_idxs=KBG, num_idxs_reg=KBG,
                        elem_size=BLOCK_SIZE * ROW_ELEMS // 2,
                        transpose=True, queue_num=0,
                    )
                    kts.append(kt)
            for g in range(ng8):
                gsz = min(GCH, N8 - g * GCH)
                toks = CHUNK * gsz
                isl = idx8[:, g * (GCH * 8) : g * (GCH * 8) + toks // 16]
                if not KBLK:
                    kt = ktp.tile([128, 4, toks], U16)
                    # transpose gathers must all share one queue: two transpose
                    # gathers in flight on different queues corrupt each other
                    nc.gpsimd.dma_gather(
                        kt[:], kc8_d[:], isl,
                        num_idxs=toks, num_idxs_reg=toks, elem_size=ROW_ELEMS // 2,
                        transpose=True, queue_num=0,
                    )
                    kts.append(kt)
                vt = vtp.tile([128, gsz, ROW_ELEMS], F8)
                nc.gpsimd.dma_gather(
                    vt[:], vc8_d[:], isl,
                    num_idxs=toks, num_idxs_reg=toks, elem_size=ROW_ELEMS,
                    transpose=False, queue_num=1 + (g % (NQ - 1)),
                )
                vts.append(vt)
            toks16 = CHUNK * N16
            kt16 = ktp16.tile([128, KVH, toks16], BF16)
            nc.gpsimd.dma_gather(
                kt16[:], kc16_d[:], idx16[:, 0 : toks16 // 16],
                num_idxs=toks16, num_idxs_reg=toks16, elem_size=ROW_ELEMS,
                transpose=True, queue_num=0,
            )
            vt16 = vtp16.tile([128, N16, ROW_ELEMS], BF16)
            nc.gpsimd.dma_gather(
                vt16[:], vc16_d[:], idx16[:, 0 : toks16 // 16],
                num_idxs=toks16, num_idxs_reg=toks16, elem_size=ROW_ELEMS,
                transpose=False, queue_num=2 % NQ,
            )

            def k_slice(c, h):
                if c < N8:
                    g, l = c // GCH, c % GCH
                    ktv = kts[g][:].bitcast(F8).rearrange("p c (t b) -> p c t b", b=2)
                    return ktv[:, h // 2, l * CHUNK : (l + 1) * CHUNK, h % 2]
                l = c - N8
                return kt16[:, h, l * CHUNK : (l + 1) * CHUNK]

            def v_slice(c, h):
                if c < N8:
                    return vts[c // GCH][:, c % GCH, h * D : (h + 1) * D]
                return vt16[:, c - N8, h * D : (h + 1) * D]

            def pv_and_den(cc, pt, po, oa, ow):
                ooc = (cc % OG) * NH
                for h in range(KVH):
                    nc.tensor.matmul(
                        oa[:, ooc + h * G : ooc + (h + 1) * G],
                        v_slice(cc, h),
                        pt[:, po + h * G : po + (h + 1) * G],
                        start=True, stop=True,
                    )
                nc.tensor.matmul(
                    oa[0:1, ow + ooc : ow + ooc + NH], ones[:],
                    pt[:, po : po + NH],
                    start=True, stop=True,
                )

            oa = scg = ptg = None
            for c in range(NTOT):
                oc = (c % OG) * NH
                if oc == 0:
                    ow = min(OG, NTOT - c) * NH
                    oa = oap.tile([128, 2 * ow], F32)  # [.., :ow]=num, [0, ow:]=den
                    if BEXP:
                        scg = scp.tile([128, ow], F32)
                sc = scg if BEXP else scp.tile([128, NH], F32)
                so = oc if BEXP else 0
                for h in range(KVH):
                    nc.tensor.matmul(
                        sc[:, so + h * G : so + (h + 1) * G],
                        k_slice(c, h),
                        qT[:, c * NH + h * G : c * NH + (h + 1) * G],
                        start=True, stop=True,
                    )
                if BEXP:
                    if oc + NH == ow:  # group end: one exp + one mask-mul + PVs
                        gbase = c - c % OG
                        ngc = ow // NH
                        pt = ptp.tile([128, ow], BF16)
                        nc.scalar.activation(
                            pt[:], sc[:], mybir.ActivationFunctionType.Exp,
                        )
                        mbc = (
                            mask[:, gbase : gbase + ngc]
                            .unsqueeze(2)
                            .broadcast_to([128, ngc, NH])
                        )
                        ptv = pt[:].rearrange("p (c q) -> p c q", q=NH)
                        nc.vector.tensor_mul(ptv, ptv, mbc)
                        for cc in range(gbase, gbase + ngc):
                            pv_and_den(cc, pt, (cc % OG) * NH, oa, ow)
                else:
                    pt = ptp.tile([128, NH], BF16)
                    nc.scalar.activation(
                        pt[:], sc[:], mybir.ActivationFunctionType.Exp,
                        bias=bias[:, c : c + 1],
                    )
                    pv_and_den(c, pt, 0, oa, ow)
                if oc + NH == ow:
                    base = (c - c % OG) * NH
                    nc.vector.tensor_copy(ost[:, base : base + ow], oa[:, 0:ow])
                    nc.vector.tensor_copy(dst[0:1, base : base + ow], oa[0:1, ow : 2 * ow])
                    nc.sync.dma_start(onum_d[:, base : base + ow], ost[:, base : base + ow])
                    nc.sync.dma_start(oden_d[:, base : base + ow], dst[0:1, base : base + ow])
    nc.finalize()
    return nc


def _plan(lens):
    """Chunk assignment. Returns (N8, N16, percore) where percore[n] is a list
    of NTOT entries (seq, chunk_idx) or None for dummies; fp8 chunks first."""
    order = np.argsort(-lens, kind="stable")
    big = [int(s) for s in order if lens[s] >= T_BF16]
    small = [int(s) for s in order if lens[s] < T_BF16]
    ch8 = [(s, c) for s in big for c in range((int(lens[s]) + CHUNK - 1) // CHUNK)]
    ch16 = [(s, c) for s in small for c in range((int(lens[s]) + CHUNK - 1) // CHUNK)]
    N8 = max(1, (len(ch8) + N_CORES - 1) // N_CORES)
    N16 = max(1, (len(ch16) + N_CORES - 1) // N_CORES)
    ch8 += [None] * (N8 * N_CORES - len(ch8))
    ch16 += [None] * (N16 * N_CORES - len(ch16))
    percore = []
    for n in range(N_CORES):
        percore.append(ch8[n * N8 : (n + 1) * N8] + ch16[n * N16 : (n + 1) * N16])
    return N8, N16, percore


def _prep(q, k, v, k_cache, v_cache, context_lens, block_tables, slot_mapping):
    lens = np.asarray(context_lens).astype(np.int64)
    bt = np.asarray(block_tables).astype(np.int64)
    sm = np.asarray(slot_mapping).astype(np.int64)

    kc = np.ascontiguousarray(np.asarray(k_cache, np.float32)).reshape(ROWS, KVH, D)
    vc = np.ascontiguousarray(np.asarray(v_cache, np.float32)).reshape(ROWS, KVH, D)
    kc = kc.copy()
    vc = vc.copy()
    kc[sm] = np.asarray(k, np.float32)
    vc[sm] = np.asarray(v, np.float32)

    # fp8 caches: K packed as u16 head-pairs, V natural
    k8 = kc.astype(F8NP)  # [ROWS, 8, 128]
    kc8 = np.ascontiguousarray(
        k8.reshape(ROWS, 4, 2, D).transpose(0, 1, 3, 2)
    ).view(np.uint8).reshape(ROWS, ROW_ELEMS).copy().view(np.uint16)
    vc8 = np.ascontiguousarray(vc.astype(F8NP).reshape(ROWS, ROW_ELEMS))
    kc16 = kc.reshape(ROWS, ROW_ELEMS).astype(ml_dtypes.bfloat16)
    vc16 = vc.reshape(ROWS, ROW_ELEMS).astype(ml_dtypes.bfloat16)

    N8, N16, percore = _plan(lens)
    NTOT = N8 + N16
    qs = (np.asarray(q, np.float32)[:, 0] * SCALE).reshape(B, NH, D)

    NBP = ((N8 * 8 + KBG - 1) // KBG) * KBG
    in_maps = []
    for n in range(N_CORES):
        idxs = np.zeros((16, NTOT * 8), np.int16)
        blkids = np.zeros(NBP, np.int16)
        qT = np.zeros((128, NTOT * NH), np.float32)
        bias = np.full((128, NTOT), -1e9, np.float32)
        for j, ch in enumerate(percore[n]):
            if ch is None:
                continue
            s, c = ch
            L = int(lens[s])
            nvalid = min(CHUNK, L - c * CHUNK)
            t0 = c * CHUNK
            nb = (min(L, t0 + CHUNK) + 15) // 16 - t0 // 16  # blocks covering chunk
            blocks = np.zeros(8, np.int64)
            blocks[:nb] = bt[s, t0 // 16 : t0 // 16 + nb]
            idxs[:, j * 8 : (j + 1) * 8] = (
                blocks[None, :] * BLOCK_SIZE + np.arange(16)[:, None]
            ).astype(np.int16)
            if j < N8:
                blkids[j * 8 : (j + 1) * 8] = blocks.astype(np.int16)
            qT[:, j * NH : (j + 1) * NH] = qs[s].reshape(NH, D).T
            bias[:nvalid, j] = 0.0
        idxb = blkids.reshape(-1, 16).T  # [16, NBP//16]
        in_maps.append(
            {
                "kc8": kc8,
                "vc8": vc8,
                "kc16": kc16,
                "vc16": vc16,
                "idx8": np.ascontiguousarray(np.tile(idxs[:, : N8 * 8], (8, 1))),
                "idxb": np.ascontiguousarray(np.tile(idxb, (8, 1))),
                "idx16": np.ascontiguousarray(np.tile(idxs[:, N8 * 8 :], (8, 1))),
                "qT": qT.astype(ml_dtypes.bfloat16),
                "bias": bias,
                "mask": (bias == 0.0).astype(ml_dtypes.bfloat16),
            }
        )
    return N8, N16, percore, in_maps


def _assemble(lens, percore, core_outs):
    lens = np.asarray(lens).astype(np.int64)
    num = np.zeros((B, NH, D), np.float64)
    den = np.zeros((B, NH), np.float64)
    for n in range(N_CORES):
        onum = np.asarray(core_outs[n]["onum"], np.float64)  # [128, NTOT*32]
        oden = np.asarray(core_outs[n]["oden"], np.float64)  # [1, NTOT*32]
        for j, ch in enumerate(percore[n]):
            if ch is None:
                continue
            s, _c = ch
            num[s] += onum[:, j * NH : (j + 1) * NH].T
            den[s] += oden[0, j * NH : (j + 1) * NH]
    out = (num / den[:, :, None]).astype(np.float32)
    return out.reshape(B, 1, NH, D)


def kernel(q, k, v, k_cache, v_cache, context_lens, block_tables, slot_mapping):
    from concourse.bass_utils import run_bass_kernel_spmd

    N8, N16, percore, in_maps = _prep(
        q, k, v, k_cache, v_cache, context_lens, block_tables, slot_mapping
    )
    key = ("hw", N8, N16)
    if key not in _prog_cache:
        _prog_cache[key] = _build_program(N8, N16)
    nc = _prog_cache[key]
    res = run_bass_kernel_spmd(nc, in_maps, list(range(N_CORES)))
    return _assemble(context_lens, percore, res.results)


# revision 4
# speedup vs baseline: 2.4980x; 2.4980x over previous
"""Paged-attention decode kernel for 8 Trainium2 NeuronCores.

Strategy vs baseline (45.26us):
  - Mixed-precision KV cache: sequences with L >= T_BF16 (=256) read K/V as
    fp8 e3m4 (half the gather bytes); shorter sequences (concentrated
    softmax, error-critical) stay bf16. Host-simulated rel err: 0.0036.
  - Exact chunk-level load balance: the global list of 128-token chunks is
    split evenly across cores (31 fp8 + 1 bf16 chunk per core for the seed-0
    shapes) instead of per-slot bucket padding (5120 -> 4096+128 tokens/core).
  - Each chunk computes an independent partial: numerator^T [128d, 32(h,q)]
    and denominator [1, 32] via per-chunk PSUM groups; host sums partials
    across chunks/cores and normalizes. No on-device reciprocal/fold.
  - Masking + padding folded into the exp bias (per-partition bias column,
    -1e9 beyond the valid token count) - no DVE mask multiply.
  - K fp8 rows are packed on host as uint16 pairs (head 2c, head 2c+1) so the
    16-bit-granular transpose gather lands K^T with d on partitions; per-head
    stationary APs are stride-2 fp8 slices. V rows gathered natural.

Layout notes:
  kc8  u16 [32768, 512]: unit u=128c+p of token row = (K[h=2c][d=p], K[h=2c+1][d=p])
  kt8 tile [128, 4, toks] u16 -> bitcast fp8 [128, 4, 2*toks]; head h chunk at
      rearrange("p c (t b) -> p c t b", b=2)[:, h//2, t0:t1, h%2]
  vt8 tile [128 tok, gsz, 1024] fp8; PV lhsT = vt[:, l, 128h:128h+128]
"""

import os
import sys
from contextlib import ExitStack

import numpy as np

for _p in ("/opt/trn_rl_repo", "/root/.axon_site/_ro/trn_rl_repo"):
    if os.path.isdir(_p) and _p not in sys.path:
        sys.path.insert(0, _p)

import ml_dtypes  # noqa: E402

import concourse.bass as bass  # noqa: E402
from concourse import bacc  # noqa: E402
import concourse.tile as tile  # noqa: E402
from concourse import mybir  # noqa: E402

B = 32
NUM_BLOCKS = 2048
BLOCK_SIZE = 16
KVH = 8
NH = 32
D = 128
MAX_BLOCKS = 128
G = NH // KVH
ROWS = NUM_BLOCKS * BLOCK_SIZE
ROW_ELEMS = KVH * D
SCALE = float(1.0 / np.sqrt(D))
N_CORES = 8
CHUNK = 128

T_BF16 = int(os.environ.get("KRN_T", "256"))
GCH = int(os.environ.get("KRN_GCH", "4"))  # chunks per gather group
OG = 4  # chunks per output psum group
NQ = int(os.environ.get("KRN_NQ", "4"))
KBLK = int(os.environ.get("KRN_KBLK", "0"))  # K block-gather: dead, walrus
# rejects multi-free-dim stationary APs ("RHS AP can only have one free dim")
BEXP = int(os.environ.get("KRN_BEXP", "1"))  # batch exp per output group +
# DVE mask multiply (instead of per-chunk exp with per-partition bias)
QUAD = int(os.environ.get("KRN_QUAD", "0"))  # K gathers fetch 2 contiguous
# token rows (2KB) per descriptor; scores split into 2 parity matmuls (bases
# 0/64) and the V/mask token order is permuted (r <-> token 2*(r%64) + r//64).
# Interleaved A/B (40 rounds): per-token K med 7.3us vs pair-K 10.6us -> off
KBG = 128  # blocks per K block-gather (16 chunks)
SCRATCH = int(os.environ.get("KRN_SCRATCH", "32768"))
F8 = mybir.dt.float8e3 if os.environ.get("KRN_F8", "e3") == "e3" else mybir.dt.float8e4
F8NP = mybir.dt.np(F8)
BF16 = mybir.dt.bfloat16
F32 = mybir.dt.float32
I16 = mybir.dt.int16
U16 = mybir.dt.uint16

_prog_cache: dict = {}


def _build_program(N8, N16, repeat=1):
    """One SPMD program for all 8 cores. N8 fp8 chunks + N16 bf16 chunks per
    core; all structure is static, all seq/mask/q data arrives via inputs."""
    NTOT = N8 + N16
    ng8 = (N8 + GCH - 1) // GCH
    assert not (KBLK and BEXP) and not KBLK, "KBLK path is dead (walrus AP limit)"

    nc = bacc.Bacc(num_swdge_queues=NQ, dynamic_dma_scratch_size=SCRATCH)
    kc8_d = nc.declare_dram_parameter("kc8", [ROWS, ROW_ELEMS // 2], U16, isOutput=False)
    vc8_d = nc.declare_dram_parameter("vc8", [ROWS, ROW_ELEMS], F8, isOutput=False)
    kc16_d = nc.declare_dram_parameter("kc16", [ROWS, ROW_ELEMS], BF16, isOutput=False)
    vc16_d = nc.declare_dram_parameter("vc16", [ROWS, ROW_ELEMS], BF16, isOutput=False)
    idx8_d = nc.declare_dram_parameter("idx8", [128, N8 * 8], I16, isOutput=False)
    NBP = ((N8 * 8 + KBG - 1) // KBG) * KBG  # padded block count (block mode)
    ng8q = (N8 + GCH - 1) // GCH
    idxb_d = nc.declare_dram_parameter(
        "idxb", [128, max(NBP // 16, ng8q * 16, 1)], I16, isOutput=False
    )
    idx16_d = nc.declare_dram_parameter("idx16", [128, N16 * 8], I16, isOutput=False)
    qT_d = nc.declare_dram_parameter("qT", [128, NTOT * NH], BF16, isOutput=False)
    bias_d = nc.declare_dram_parameter("bias", [128, NTOT], F32, isOutput=False)
    mask_d = nc.declare_dram_parameter("mask", [128, NTOT], BF16, isOutput=False)
    onum_d = nc.declare_dram_parameter("onum", [128, NTOT * NH], F32, isOutput=True)
    oden_d = nc.declare_dram_parameter("oden", [1, NTOT * NH], F32, isOutput=True)

    with tile.TileContext(nc) as tc, ExitStack() as ctx:
        const = ctx.enter_context(tc.tile_pool(name="const", bufs=1))
        ktp = ctx.enter_context(tc.tile_pool(name="ktp", bufs=(NBP // KBG) if KBLK else max(2, ng8)))
        vtp = ctx.enter_context(tc.tile_pool(name="vtp", bufs=max(2, ng8)))
        ktp16 = ctx.enter_context(tc.tile_pool(name="ktp16", bufs=2))
        vtp16 = ctx.enter_context(tc.tile_pool(name="vtp16", bufs=2))
        ptp = ctx.enter_context(tc.tile_pool(name="ptp", bufs=4))
        scp = ctx.enter_context(tc.tile_pool(name="scp", bufs=4, space=bass.MemorySpace.PSUM))
        oap = ctx.enter_context(tc.tile_pool(name="oap", bufs=4, space=bass.MemorySpace.PSUM))

        idx8 = const.tile([128, N8 * 8], I16)
        nc.sync.dma_start(idx8[:], idx8_d[:])
        idxb = const.tile([128, max(NBP // 16, ng8q * 16, 1)], I16)
        if KBLK or QUAD:
            nc.sync.dma_start(idxb[:], idxb_d[:])
        idx16 = const.tile([128, N16 * 8], I16)
        nc.sync.dma_start(idx16[:], idx16_d[:])
        qT = const.tile([128, NTOT * NH], BF16)
        nc.sync.dma_start(qT[:], qT_d[:])
        bias = const.tile([128, NTOT], F32)
        mask = const.tile([128, NTOT], BF16)
        if BEXP:
            nc.sync.dma_start(mask[:], mask_d[:])
        else:
            nc.sync.dma_start(bias[:], bias_d[:])
        ones = const.tile([128, 1], BF16)
        nc.vector.memset(ones[:], 1.0)
        ost = const.tile([128, NTOT * NH], F32)
        dst = const.tile([1, NTOT * NH], F32)

        for _rep in range(repeat):
            kts, vts = [], []
            if QUAD:
                kcq = kc8_d[:].rearrange("(q t) e -> q (t e)", t=2)
                for g in range(ng8):
                    kt = ktp.tile([128, 8, 256], U16)
                    nc.gpsimd.dma_gather(
                        kt[:], kcq, idxb[:, g * 16 : g * 16 + 16],
                        num_idxs=256, num_idxs_reg=256,
                        elem_size=2 * ROW_ELEMS // 2,
                        transpose=True, queue_num=0,
                    )
                    kts.append(kt)
            if KBLK:
                # K at block granularity: 128-block transpose gathers, 16KB
                # contiguous per descriptor (a block's 16 token rows).
                kcb = kc8_d[:].rearrange("(b t) e -> b (t e)", t=BLOCK_SIZE)
                for kb in range(NBP // KBG):
                    kt = ktp.tile([128, (KBG * BLOCK_SIZE * 4) // KBG, KBG], U16)
                    nc.gpsimd.dma_gather(
                        kt[:], kcb, idxb[:, kb * (KBG // 16) : (kb + 1) * (KBG // 16)],
                        num_idxs=KBG, num_idxs_reg=KBG,
                        elem_size=BLOCK_SIZE * ROW_ELEMS // 2,
                        transpose=True, queue_num=0,
                    )
                    kts.append(kt)
            for g in range(ng8):
                gsz = min(GCH, N8 - g * GCH)
                toks = CHUNK * gsz
                isl = idx8[:, g * (GCH * 8) : g * (GCH * 8) + toks // 16]
                if not KBLK and not QUAD:
                    kt = ktp.tile([128, 4, toks], U16)
                    # transpose gathers must all share one queue: two transpose
                    # gathers in flight on different queues corrupt each other
                    nc.gpsimd.dma_gather(
                        kt[:], kc8_d[:], isl,
                        num_idxs=toks, num_idxs_reg=toks, elem_size=ROW_ELEMS // 2,
                        transpose=True, queue_num=0,
                    )
                    kts.append(kt)
                vt = vtp.tile([128, gsz, ROW_ELEMS], F8)
                nc.gpsimd.dma_gather(
                    vt[:], vc8_d[:], isl,
                    num_idxs=toks, num_idxs_reg=toks, elem_size=ROW_ELEMS,
                    transpose=False, queue_num=1 + (g % (NQ - 1)),
                )
                vts.append(vt)
            toks16 = CHUNK * N16
            kt16 = ktp16.tile([128, KVH, toks16], BF16)
            nc.gpsimd.dma_gather(
                kt16[:], kc16_d[:], idx16[:, 0 : toks16 // 16],
                num_idxs=toks16, num_idxs_reg=toks16, elem_size=ROW_ELEMS,
                transpose=True, queue_num=0,
            )
            vt16 = vtp16.tile([128, N16, ROW_ELEMS], BF16)
            nc.gpsimd.dma_gather(
                vt16[:], vc16_d[:], idx16[:, 0 : toks16 // 16],
                num_idxs=toks16, num_idxs_reg=toks16, elem_size=ROW_ELEMS,
                transpose=False, queue_num=2 % NQ,
            )

            def k_slice(c, h):
                if c < N8:
                    g, l = c // GCH, c % GCH
                    ktv = kts[g][:].bitcast(F8).rearrange("p c (t b) -> p c t b", b=2)
                    return ktv[:, h // 2, l * CHUNK : (l + 1) * CHUNK, h % 2]
                l = c - N8
                return kt16[:, h, l * CHUNK : (l + 1) * CHUNK]

            def k_slice_quad(c, h, tp):
                # kt [128, 8, 256] u16 -> fp8 [128, 8, 512]; chunk-local l,
                # parity tp, head h: cc = 4*tp + h//2, last = 2*(64l + j) + h%2
                g, l = c // GCH, c % GCH
                kf = kts[g][:].bitcast(F8)
                return kf[:, 4 * tp + h // 2, 128 * l + (h % 2) : 128 * l + 127 + (h % 2) : 2]

            def v_slice(c, h):
                if c < N8:
                    return vts[c // GCH][:, c % GCH, h * D : (h + 1) * D]
                return vt16[:, c - N8, h * D : (h + 1) * D]

            def pv_and_den(cc, pt, po, oa, ow):
                ooc = (cc % OG) * NH
                for h in range(KVH):
                    nc.tensor.matmul(
                        oa[:, ooc + h * G : ooc + (h + 1) * G],
                        v_slice(cc, h),
                        pt[:, po + h * G : po + (h + 1) * G],
                        start=True, stop=True,
                    )
                nc.tensor.matmul(
                    oa[0:1, ow + ooc : ow + ooc + NH], ones[:],
                    pt[:, po : po + NH],
                    start=True, stop=True,
                )

            oa = scg = ptg = None
            for c in range(NTOT):
                oc = (c % OG) * NH
                if oc == 0:
                    ow = min(OG, NTOT - c) * NH
                    oa = oap.tile([128, 2 * ow], F32)  # [.., :ow]=num, [0, ow:]=den
                    if BEXP:
                        scg = scp.tile([128, ow], F32)
                sc = scg if BEXP else scp.tile([128, NH], F32)
                so = oc if BEXP else 0
                for h in range(KVH):
                    if QUAD and c < N8:
                        for tp in range(2):
                            nc.tensor.matmul(
                                sc[64 * tp : 64 * tp + 64, so + h * G : so + (h + 1) * G],
                                k_slice_quad(c, h, tp),
                                qT[:, c * NH + h * G : c * NH + (h + 1) * G],
                                start=True, stop=True,
                            )
                    else:
                        nc.tensor.matmul(
                            sc[:, so + h * G : so + (h + 1) * G],
                            k_slice(c, h),
                            qT[:, c * NH + h * G : c * NH + (h + 1) * G],
                            start=True, stop=True,
                        )
                if BEXP:
                    if oc + NH == ow:  # group end: one exp + one mask-mul + PVs
                        gbase = c - c % OG
                        ngc = ow // NH
                        pt = ptp.tile([128, ow], BF16)
                        nc.scalar.activation(
                            pt[:], sc[:], mybir.ActivationFunctionType.Exp,
                        )
                        mbc = (
                            mask[:, gbase : gbase + ngc]
                            .unsqueeze(2)
                            .broadcast_to([128, ngc, NH])
                        )
                        ptv = pt[:].rearrange("p (c q) -> p c q", q=NH)
                        nc.vector.tensor_mul(ptv, ptv, mbc)
                        for cc in range(gbase, gbase + ngc):
                            pv_and_den(cc, pt, (cc % OG) * NH, oa, ow)
                else:
                    pt = ptp.tile([128, NH], BF16)
                    nc.scalar.activation(
                        pt[:], sc[:], mybir.ActivationFunctionType.Exp,
                        bias=bias[:, c : c + 1],
                    )
                    pv_and_den(c, pt, 0, oa, ow)
                if oc + NH == ow:
                    base = (c - c % OG) * NH
                    nc.vector.tensor_copy(ost[:, base : base + ow], oa[:, 0:ow])
                    nc.vector.tensor_copy(dst[0:1, base : base + ow], oa[0:1, ow : 2 * ow])
                    nc.sync.dma_start(onum_d[:, base : base + ow], ost[:, base : base + ow])
                    nc.sync.dma_start(oden_d[:, base : base + ow], dst[0:1, base : base + ow])
    nc.finalize()
    return nc


def _plan(lens):
    """Chunk assignment. Returns (N8, N16, percore) where percore[n] is a list
    of NTOT entries (seq, chunk_idx) or None for dummies; fp8 chunks first."""
    order = np.argsort(-lens, kind="stable")
    big = [int(s) for s in order if lens[s] >= T_BF16]
    small = [int(s) for s in order if lens[s] < T_BF16]
    ch8 = [(s, c) for s in big for c in range((int(lens[s]) + CHUNK - 1) // CHUNK)]
    ch16 = [(s, c) for s in small for c in range((int(lens[s]) + CHUNK - 1) // CHUNK)]
    N8 = max(1, (len(ch8) + N_CORES - 1) // N_CORES)
    N16 = max(1, (len(ch16) + N_CORES - 1) // N_CORES)
    ch8 += [None] * (N8 * N_CORES - len(ch8))
    ch16 += [None] * (N16 * N_CORES - len(ch16))
    percore = []
    for n in range(N_CORES):
        percore.append(ch8[n * N8 : (n + 1) * N8] + ch16[n * N16 : (n + 1) * N16])
    return N8, N16, percore


def _prep(q, k, v, k_cache, v_cache, context_lens, block_tables, slot_mapping):
    lens = np.asarray(context_lens).astype(np.int64)
    bt = np.asarray(block_tables).astype(np.int64)
    sm = np.asarray(slot_mapping).astype(np.int64)

    kc = np.ascontiguousarray(np.asarray(k_cache, np.float32)).reshape(ROWS, KVH, D)
    vc = np.ascontiguousarray(np.asarray(v_cache, np.float32)).reshape(ROWS, KVH, D)
    kc = kc.copy()
    vc = vc.copy()
    kc[sm] = np.asarray(k, np.float32)
    vc[sm] = np.asarray(v, np.float32)

    # fp8 caches: K packed as u16 head-pairs, V natural
    k8 = kc.astype(F8NP)  # [ROWS, 8, 128]
    kc8 = np.ascontiguousarray(
        k8.reshape(ROWS, 4, 2, D).transpose(0, 1, 3, 2)
    ).view(np.uint8).reshape(ROWS, ROW_ELEMS).copy().view(np.uint16)
    vc8 = np.ascontiguousarray(vc.astype(F8NP).reshape(ROWS, ROW_ELEMS))
    kc16 = kc.reshape(ROWS, ROW_ELEMS).astype(ml_dtypes.bfloat16)
    vc16 = vc.reshape(ROWS, ROW_ELEMS).astype(ml_dtypes.bfloat16)

    N8, N16, percore = _plan(lens)
    NTOT = N8 + N16
    qs = (np.asarray(q, np.float32)[:, 0] * SCALE).reshape(B, NH, D)

    NBP = ((N8 * 8 + KBG - 1) // KBG) * KBG
    in_maps = []
    for n in range(N_CORES):
        idxs = np.zeros((16, NTOT * 8), np.int16)
        blkids = np.zeros(NBP, np.int16)
        qT = np.zeros((128, NTOT * NH), np.float32)
        bias = np.full((128, NTOT), -1e9, np.float32)
        quadids = np.zeros(((N8 + GCH - 1) // GCH) * 256, np.int16)
        for j, ch in enumerate(percore[n]):
            if ch is None:
                continue
            s, c = ch
            L = int(lens[s])
            nvalid = min(CHUNK, L - c * CHUNK)
            t0 = c * CHUNK
            nb = (min(L, t0 + CHUNK) + 15) // 16 - t0 // 16  # blocks covering chunk
            blocks = np.zeros(8, np.int64)
            blocks[:nb] = bt[s, t0 // 16 : t0 // 16 + nb]
            toklist = (blocks[:, None] * BLOCK_SIZE + np.arange(16)[None, :]).reshape(-1)
            valid = np.zeros(CHUNK, np.float32)
            valid[:nvalid] = 1.0
            if QUAD and j < N8:
                # V row r <- chunk token 2*(r%64) + r//64 (pair-split score rows)
                perm = 2 * (np.arange(CHUNK) % 64) + np.arange(CHUNK) // 64
                toklist = toklist[perm]
                valid = valid[perm]
                # K pair ids: position i = 64*l + q -> block q//8, pair q%8
                qq = np.arange(64)
                quadids[j * 64 : (j + 1) * 64] = (
                    blocks[qq // 8] * 8 + qq % 8
                ).astype(np.int16)
            idxs[:, j * 8 : (j + 1) * 8] = (
                toklist.reshape(8, 16).T
            ).astype(np.int16)
            if j < N8:
                blkids[j * 8 : (j + 1) * 8] = blocks.astype(np.int16)
            qT[:, j * NH : (j + 1) * NH] = qs[s].reshape(NH, D).T
            bias[valid > 0, j] = 0.0
        W = max(NBP // 16, ((N8 + GCH - 1) // GCH) * 16, 1)
        idxb = np.zeros((16, W), np.int16)
        src = quadids if QUAD else blkids
        wrap = src.reshape(-1, 16).T
        idxb[:, : wrap.shape[1]] = wrap
        in_maps.append(
            {
                "kc8": kc8,
                "vc8": vc8,
                "kc16": kc16,
                "vc16": vc16,
                "idx8": np.ascontiguousarray(np.tile(idxs[:, : N8 * 8], (8, 1))),
                "idxb": np.ascontiguousarray(np.tile(idxb, (8, 1))),
                "idx16": np.ascontiguousarray(np.tile(idxs[:, N8 * 8 :], (8, 1))),
                "qT": qT.astype(ml_dtypes.bfloat16),
                "bias": bias,
                "mask": (bias == 0.0).astype(ml_dtypes.bfloat16),
            }
        )
    return N8, N16, percore, in_maps


def _assemble(lens, percore, core_outs):
    lens = np.asarray(lens).astype(np.int64)
    num = np.zeros((B, NH, D), np.float64)
    den = np.zeros((B, NH), np.float64)
    for n in range(N_CORES):
        onum = np.asarray(core_outs[n]["onum"], np.float64)  # [128, NTOT*32]
        oden = np.asarray(core_outs[n]["oden"], np.float64)  # [1, NTOT*32]
        for j, ch in enumerate(percore[n]):
            if ch is None:
                continue
            s, _c = ch
            num[s] += onum[:, j * NH : (j + 1) * NH].T
            den[s] += oden[0, j * NH : (j + 1) * NH]
    out = (num / den[:, :, None]).astype(np.float32)
    return out.reshape(B, 1, NH, D)


def kernel(q, k, v, k_cache, v_cache, context_lens, block_tables, slot_mapping):
    from concourse.bass_utils import run_bass_kernel_spmd

    N8, N16, percore, in_maps = _prep(
        q, k, v, k_cache, v_cache, context_lens, block_tables, slot_mapping
    )
    key = ("hw", N8, N16)
    if key not in _prog_cache:
        _prog_cache[key] = _build_program(N8, N16)
    nc = _prog_cache[key]
    res = run_bass_kernel_spmd(nc, in_maps, list(range(N_CORES)))
    return _assemble(context_lens, percore, res.results)
